# revision 8
# baseline (speedup 1.0000x reference)
"""CBAM attention Trainium2 kernel.

Full inputs: x [32, 256, 64, 64], w1 [16, 256], w2 [256, 16], ws [1, 2, 7, 7].
Data-parallel across 8 NeuronCores: 4 samples per core, weights replicated.

Per-core layout: channels on partitions (2 tiles of 128 per sample), spatial
(64*64=4096) along the free dim.
"""

import numpy as np

B, C, H, W = 32, 256, 64, 64
NCORES = 8
B_LOC = B // NCORES          # 4 samples per core
MID = 16
HW = H * W                   # 4096
NT = C // 128                # 2 channel tiles
PW = 70                      # padded row width (W + 2*3)
PROW = PW * PW               # 4900 padded plane size

_cached_nc = None


def _build():
    from concourse import bacc, tile
    import concourse.mybir as mybir

    F32 = mybir.dt.float32
    AF = mybir.ActivationFunctionType
    AX = mybir.AxisListType
    ALU = mybir.AluOpType

    nc = bacc.Bacc("TRN2", target_bir_lowering=False, debug=False,
                   num_devices=NCORES)

    x_d = nc.dram_tensor("x", [B_LOC, C, H, W], F32, kind="ExternalInput")
    ident_d = nc.dram_tensor("ident", [128, 128], F32, kind="ExternalInput")
    ones_d = nc.dram_tensor("ones", [128, 1], F32, kind="ExternalInput")
    w1t_d = nc.dram_tensor("w1t", [128, NT, MID], F32, kind="ExternalInput")
    w2t_d = nc.dram_tensor("w2t", [MID, C], F32, kind="ExternalInput")
    wconv_d = nc.dram_tensor("wconv", [98, 128], F32, kind="ExternalInput")
    out_d = nc.dram_tensor("out", [B_LOC, C, H, W], F32, kind="ExternalOutput")

    with tile.TileContext(nc) as tc:
        with (
            tc.tile_pool(name="xs", bufs=1) as xpool,
            tc.tile_pool(name="work", bufs=1) as work,
            tc.tile_pool(name="imc", bufs=1) as imcpool,
            tc.tile_pool(name="grep", bufs=1) as gpool,
            tc.tile_pool(name="sm", bufs=2) as smpool,
            tc.tile_pool(name="tp", bufs=2, space="PSUM") as tppool,
            tc.tile_pool(name="p4", bufs=2, space="PSUM") as p4pool,
            tc.tile_pool(name="gps", bufs=2, space="PSUM") as gpspool,
            tc.tile_pool(name="mlp", bufs=1, space="PSUM") as mlppool,
        ):
            # ---- constants ----
            ident = work.tile([128, 128], F32, tag="ident")
            ones = work.tile([128, 1], F32, tag="ones")
            w1t = work.tile([128, NT, MID], F32, tag="w1t")
            w2t = work.tile([MID, C], F32, tag="w2t")
            wconv = work.tile([98, 128], F32, tag="wconv")
            nc.sync.dma_start(ident[:], ident_d.ap())
            nc.sync.dma_start(ones[:], ones_d.ap())
            nc.sync.dma_start(w1t[:], w1t_d.ap())
            nc.sync.dma_start(w2t[:], w2t_d.ap())
            nc.sync.dma_start(wconv[:], wconv_d.ap())

            # ---- working buffers ----
            # stat cols: 2*(t*4+b) = avg, 2*(t*4+b)+1 = max ; 16 + t*4+b = ca
            # cols 24, 25: partial-sum scratch
            stat = work.tile([128, 26], F32, tag="stat")
            hs = work.tile([MID, 3 * B_LOC], F32, tag="hs")
            junk = work.tile([128, HW // 2], F32, tag="junk")
            featpad = work.tile([2, PROW], F32, tag="featpad")
            feat_dx = work.tile([14, PROW], F32, tag="feat_dx")

            xs = [[xpool.tile([128, HW], F32, tag=f"x{b}_{t}",
                              name=f"x{b}_{t}")
                   for t in range(NT)] for b in range(B_LOC)]

            nc.vector.memset(featpad[:], 0.0)

            fpv = featpad[:].rearrange("p (y x) -> p y x", y=PW, x=PW)
            fdv = feat_dx[:].rearrange("p (y x) -> p y x", y=PW, x=PW)

            for b in range(B_LOC):
                # ---- load x ----
                for t in range(NT):
                    src = x_d.ap()[b][t * 128:(t + 1) * 128]
                    nc.sync.dma_start(
                        xs[b][t][:], src.rearrange("c h w -> c (h w)"))

                # ---- channel attention pooling ----
                for t in range(NT):
                    j = t * B_LOC + b
                    # spatial mean via ACT accumulate (junk main output)
                    for hh in range(2):
                        nc.scalar.activation(
                            junk[:],
                            xs[b][t][:, hh * (HW // 2):(hh + 1) * (HW // 2)],
                            AF.Copy, scale=1.0 / HW,
                            accum_out=stat[:, 24 + hh:25 + hh])
                    nc.vector.tensor_add(
                        stat[:, 2 * j:2 * j + 1], stat[:, 24:25],
                        stat[:, 25:26])
                    # spatial max on DVE
                    nc.vector.reduce_max(
                        stat[:, 2 * j + 1:2 * j + 2], xs[b][t][:], axis=AX.X)

                # ---- MLP: h = relu(w1 @ [avg, max]) ; o = w2 @ h ----
                hp = mlppool.tile([MID, 2], F32, tag="mlp")
                for t in range(NT):
                    j = t * B_LOC + b
                    nc.tensor.matmul(
                        hp[:], w1t[:, t, :], stat[:, 2 * j:2 * j + 2],
                        start=(t == 0), stop=(t == NT - 1))
                # relu both columns; accum_out gives relu_avg + relu_max
                hsum = hs[:, 2 * B_LOC + b:2 * B_LOC + b + 1]
                nc.scalar.activation(hs[:, 2 * b:2 * b + 2], hp[:], AF.Relu,
                                     accum_out=hsum)
                for t in range(NT):
                    op = mlppool.tile([128, 1], F32, tag="mlp",
                                      name=f"op{b}_{t}")
                    nc.tensor.matmul(
                        op[:], w2t[:, t * 128:(t + 1) * 128],
                        hsum, start=True, stop=True)
                    cacol = stat[:, 16 + t * B_LOC + b:17 + t * B_LOC + b]
                    nc.scalar.activation(cacol, op[:], AF.Sigmoid)

                # ---- xc = x * ca (in place, per-partition scale) ----
                for t in range(NT):
                    cacol = stat[:, 16 + t * B_LOC + b:17 + t * B_LOC + b]
                    nc.scalar.activation(
                        xs[b][t][:], xs[b][t][:], AF.Copy, scale=cacol)

                # ---- spatial attention: channel sum (PE) -> feat row 0 ----
                for jc in range(8):
                    p4 = p4pool.tile([1, 512], F32, tag="p4")
                    for t in range(NT):
                        nc.tensor.matmul(
                            p4[:], ones[:],
                            xs[b][t][:, jc * 512:(jc + 1) * 512],
                            start=(t == 0), stop=(t == NT - 1))
                    dst = fpv[0:1, 3 + 8 * jc:3 + 8 * jc + 8, 3:3 + W]
                    nc.vector.tensor_copy(
                        dst, p4[0:1, :].rearrange("p (y x) -> p y x", y=8, x=W))

                # ---- spatial attention: channel max (PE transpose + DVE) ----
                samax = smpool.tile([128, 32], F32, tag="samax")
                for g in range(16):
                    tp = tppool.tile([128, 2, 256], F32, tag="tp")
                    for cc in range(2):
                        for t in range(NT):
                            nc.tensor.transpose(
                                tp[:, cc, t * 128:(t + 1) * 128],
                                xs[b][t][:, (2 * g + cc) * 128:
                                         (2 * g + cc + 1) * 128],
                                ident[:])
                    nc.vector.reduce_max(
                        samax[:, 2 * g:2 * g + 2], tp[:], axis=AX.X)
                # transpose [128, 32] -> [32, 128] to reach free-layout
                smp = mlppool.tile([32, 128], F32, tag="mlp")
                nc.tensor.transpose(smp[:], samax[:], ident[:])
                samaxT = smpool.tile([32, 128], F32, tag="samaxT")
                nc.scalar.activation(samaxT[:], smp[:], AF.Copy)
                # scatter into feat row 1 (hw = k*128 + e ; y = 2k + e//64)
                for y1 in range(2):
                    nc.gpsimd.dma_start(
                        fpv[1:2, 3 + y1:3 + y1 + 64:2, 3:3 + W],
                        samaxT[:, y1 * W:(y1 + 1) * W])

                # ---- im2col stage A: dx-shifted copies ----
                ff = featpad[:]
                for c in range(2):
                    for dx in range(7):
                        nc.gpsimd.dma_start(
                            feat_dx[c * 7 + dx:c * 7 + dx + 1, 0:PROW - 6],
                            ff[c:c + 1, dx:dx + PROW - 6])
                # ---- halves: im2col stage B + conv + sigmoid + multiply ----
                for hh in range(2):
                    imc = imcpool.tile([98, HW // 2], F32, tag="imc",
                                       name=f"imc{b}_{hh}")
                    imv = imc[:].rearrange("p (y x) -> p y x", y=H // 2, x=W)
                    for c in range(2):
                        for dy in range(7):
                            nc.gpsimd.dma_start(
                                imv[c * 49 + dy * 7:c * 49 + dy * 7 + 7,
                                    :, :],
                                fdv[c * 7:(c + 1) * 7,
                                    hh * (H // 2) + dy:
                                    hh * (H // 2) + dy + H // 2, 0:W])

                    # conv (PE, weights replicated across 128 cols) + sigmoid
                    grep = gpool.tile([128, HW // 2], F32, tag="grep",
                                      name=f"grep{b}_{hh}")
                    for jc in range(4):
                        gp = gpspool.tile([128, 512], F32, tag="gps",
                                          name=f"gps{b}_{hh}_{jc}")
                        nc.tensor.matmul(
                            gp[:], wconv[:], imc[:, jc * 512:(jc + 1) * 512],
                            start=True, stop=True)
                        nc.scalar.activation(
                            grep[:, jc * 512:(jc + 1) * 512], gp[:],
                            AF.Sigmoid)

                    # out = xc * g (in place)
                    for t in range(NT):
                        nc.vector.tensor_mul(
                            xs[b][t][:, hh * (HW // 2):(hh + 1) * (HW // 2)],
                            xs[b][t][:, hh * (HW // 2):(hh + 1) * (HW // 2)],
                            grep[:])

                # ---- store ----
                for t in range(NT):
                    dst = out_d.ap()[b][t * 128:(t + 1) * 128]
                    nc.sync.dma_start(
                        dst.rearrange("c h w -> c (h w)"), xs[b][t][:])

    nc.compile()
    return nc


def _host_consts(w1, w2, ws):
    ident = np.eye(128, dtype=np.float32)
    ones = np.ones((128, 1), np.float32)
    # w1 [MID, C] -> lhsT layout [128, NT, MID]
    w1t = np.ascontiguousarray(
        w1.T.reshape(NT, 128, MID).transpose(1, 0, 2)).astype(np.float32)
    w2t = np.ascontiguousarray(w2.T).astype(np.float32)      # [MID, C]
    wf = np.asarray(ws, np.float32)[0]                       # [2, 7, 7]
    wcol = np.empty((98, 1), np.float32)
    wcol[:49, 0] = (wf[0] / C).reshape(-1)
    wcol[49:, 0] = wf[1].reshape(-1)
    wconv = np.repeat(wcol, 128, axis=1).astype(np.float32)
    return ident, ones, w1t, w2t, wconv


def kernel(x, w1, w2, ws):
    global _cached_nc
    from concourse.bass_utils import run_bass_kernel_spmd

    if _cached_nc is None:
        _cached_nc = _build()
    nc = _cached_nc

    x = np.asarray(x, np.float32)
    ident, ones, w1t, w2t, wconv = _host_consts(w1, w2, ws)
    in_maps = []
    for i in range(NCORES):
        in_maps.append({
            "x": np.ascontiguousarray(x[i * B_LOC:(i + 1) * B_LOC]),
            "ident": ident, "ones": ones, "w1t": w1t, "w2t": w2t,
            "wconv": wconv,
        })
    res = run_bass_kernel_spmd(nc, in_maps, core_ids=list(range(NCORES)))
    out = np.concatenate([res.results[i]["out"] for i in range(NCORES)],
                         axis=0)
    return out.astype(np.float32)
